# revision 5
# baseline (speedup 1.0000x reference)
"""3D Haar DWT (depth-1) Trainium2 kernel.

Full inputs: x [4, 4, 64, 256, 256] f32 + six banded Haar matrices
(hardcoded math: every output element is +-2^-1.5 times a +-sum of a
2x2x2 block). Returns the 8 subbands (LLL, LLH, LHL, LHH, HLL, HLH,
HHL, HHH), each [4, 4, 32, 128, 128] f32.

Sharding: data-parallel over N*C = 16 sample-channels, 2 per core on
8 cores. Per-core compute is a 3-stage butterfly over pair-packed
tiles (SBUF partition p holds input rows 2p and 2p+1 contiguously, so
every DMA descriptor is a 2 KiB linear run):
  H stage: TensorE float32r matmuls against +-2^-1.5 * I (1 cyc/row)
  evac:    ScalarE PSUM->SBUF fp16 copy that also de-interleaves
           even/odd w columns, so later DVE ops see packed fp16
  W stage: DVE fp16 tensor_add/sub, packed last dim -> 2x mode
  D stage: DVE fp16 tensor_add/sub on the two d-slices of the pair
Output is written fp16 (tolerance is 2e-2; fp16 error ~1e-3) which
halves the DMA write traffic; host upcasts to f32.
"""
import sys

sys.path.insert(0, "/opt/trn_rl_repo")

import numpy as np

N, C, D, H, W = 4, 4, 64, 256, 256
NCORES = 8
G_PER_CORE = (N * C) // NCORES        # 2
KP = D // 2                           # 32 d-pairs per g
KB = 8                                # k-slices per output staging block
S3 = float(2.0 ** -1.5)

IN_BUFS = 12
WE_BUFS = 4
WT_BUFS = 4
OS_BUFS = 2
PSUM_BUFS = 3

_CACHE = {}


def _build_filter_lhst():
    """Stationary operands: +S3*I and -S3*I, as [2, 128, 128] fp32."""
    eye = np.eye(128, dtype=np.float32)
    return np.stack([np.float32(S3) * eye, np.float32(-S3) * eye])


def _build_nc():
    import concourse.bass as bass
    import concourse.tile as tile
    from concourse import bacc, mybir

    f32 = mybir.dt.float32
    f32r = mybir.dt.float32r
    f16 = mybir.dt.float16
    nc = bacc.Bacc(None)
    x_d = nc.declare_dram_parameter("x", [G_PER_CORE, D, H, W], f32r,
                                    isOutput=False)
    ft_d = nc.declare_dram_parameter("ft", [2, 128, 128], f32r,
                                     isOutput=False)
    # h'-major fp16 layout: per (band, g, partition=h') a k-block of 8 is
    # one contiguous 2 KiB run in DRAM (host transposes k and h' back)
    o_d = nc.declare_dram_parameter("out", [8, G_PER_CORE, 128, KP, 128],
                                    f16, isOutput=True)

    with tile.TileContext(nc) as tc:
        with (
            tc.tile_pool(name="cst", bufs=1) as cst,
            tc.tile_pool(name="inp", bufs=IN_BUFS) as inp,
            tc.tile_pool(name="we", bufs=WE_BUFS) as wep,
            tc.tile_pool(name="wt", bufs=WT_BUFS) as wtp,
            tc.tile_pool(name="os", bufs=OS_BUFS) as osp,
            tc.tile_pool(name="ps", bufs=PSUM_BUFS, space="PSUM") as psp,
        ):
            ft = cst.tile([128, 256], f32r, tag="ft")
            nc.sync.dma_start(
                ft.rearrange("p (i c) -> p i c", i=2),
                ft_d.rearrange("i p c -> p i c"))
            pos_i = ft[:, 0:128]    # +S3 * I
            neg_i = ft[:, 128:256]  # -S3 * I

            for g in range(G_PER_CORE):
                for kb in range(KP // KB):
                    os_t = osp.tile([128, 8 * KB * 128], f16, tag="os")
                    # [p, band(8), k8(KB), w'(128)]
                    ot3 = os_t.rearrange("p (b q w) -> p b q w", b=8, q=KB)
                    for k8 in range(KB):
                        k = kb * KB + k8
                        # one d-pair as a pair-packed tile [128, 1024]:
                        # cols = {s0: row2p row2p+1 | s1: row2p row2p+1}
                        t = inp.tile([128, 1024], f32r, tag="xin")
                        eng = nc.sync if (k % 2 == 0) else nc.scalar
                        eng.dma_start(
                            t.rearrange("p (s r) -> p s r", s=2),
                            x_d[g, 2 * k:2 * k + 2].rearrange(
                                "s (p r) w -> p s (r w)", r=2))
                        t4 = t.rearrange("p (s r w) -> p s r w", s=2, r=2)

                        # --- H stage: float32r matmuls vs +-S3*I ---
                        pt = psp.tile([128, 1024], f32, tag="ps")
                        lo = pt[:, 0:512].rearrange(
                            "p (s w) -> p s w", s=2)
                        hi = pt[:, 512:1024].rearrange(
                            "p (s w) -> p s w", s=2)
                        x0 = t4[:, :, 0, :]
                        x1 = t4[:, :, 1, :]
                        nc.tensor.matmul(lo, pos_i, x0,
                                         start=True, stop=False)
                        nc.tensor.matmul(hi, pos_i, x0,
                                         start=True, stop=False)
                        nc.tensor.matmul(lo, pos_i, x1,
                                         start=False, stop=True)
                        nc.tensor.matmul(hi, neg_i, x1,
                                         start=False, stop=True)

                        # --- PSUM evac on ScalarE: f32 -> fp16 and
                        # de-interleave w parity for packed DVE reads ---
                        # WE: [p, lh(2), s(2), q(2 parity), w'(128)]
                        we_t = wep.tile([128, 1024], f16, tag="we")
                        we5 = we_t.rearrange("p (l s q w) -> p l s q w",
                                             l=2, s=2, q=2)
                        nc.scalar.activation(
                            we_t.rearrange("p (m q w) -> p m q w",
                                           m=4, q=2),
                            pt.rearrange("p (m w q) -> p m q w",
                                         m=4, q=2),
                            mybir.ActivationFunctionType.Copy)

                        # --- W stage on DVE, fp16 2x ---
                        # WT: [p, c(4 = lh*2+wp), s(2), w'(128)]
                        wt_t = wtp.tile([128, 1024], f16, tag="wt")
                        wt4 = wt_t.rearrange("p (c s w) -> p c s w",
                                             c=4, s=2)
                        in0 = we5[:, :, :, 0, :]
                        in1 = we5[:, :, :, 1, :]
                        nc.vector.tensor_add(wt4[:, 0::2], in0, in1)
                        nc.vector.tensor_sub(wt4[:, 1::2], in0, in1)

                        # --- D stage on DVE, fp16 2x ---
                        # band = dp*4 + c  (matches reference order)
                        s0 = wt4[:, :, 0, :]
                        s1 = wt4[:, :, 1, :]
                        nc.vector.tensor_add(ot3[:, 0:4, k8], s0, s1)
                        nc.vector.tensor_sub(ot3[:, 4:8, k8], s0, s1)

                    # --- store this k-block: 8 subbands x [128,KB,128].
                    # Issued from the (otherwise idle) GpSimd sequencer so
                    # they never block input prefetch on the Sync engine.
                    for b in range(8):
                        nc.gpsimd.dma_start(
                            o_d[b, g, :, kb * KB:(kb + 1) * KB, :],
                            ot3[:, b])
    nc.finalize()
    return nc


def _get_nc():
    if "nc" not in _CACHE:
        _CACHE["nc"] = _build_nc()
    return _CACHE["nc"]


def kernel(x, low_0, low_1, low_2, high_0, high_1, high_2):
    from concourse.bass_utils import run_bass_kernel_spmd

    x = np.ascontiguousarray(np.asarray(x, dtype=np.float32))
    ft = _build_filter_lhst()
    xs = x.reshape(N * C, D, H, W)
    in_maps = [
        {"x": np.ascontiguousarray(
            xs[c * G_PER_CORE:(c + 1) * G_PER_CORE]), "ft": ft}
        for c in range(NCORES)
    ]
    nc = _get_nc()
    res = run_bass_kernel_spmd(nc, in_maps, list(range(NCORES)))
    full = np.empty((8, N * C, KP, 128, 128), dtype=np.float32)
    for c in range(NCORES):
        full[:, c * G_PER_CORE:(c + 1) * G_PER_CORE] = \
            res.results[c]["out"].transpose(0, 1, 3, 2, 4).astype(
                np.float32)
    full = full.reshape(8, N, C, KP, 128, 128)
    return tuple(full[s] for s in range(8))


# revision 6
# speedup vs baseline: 1.0567x; 1.0567x over previous
"""3D Haar DWT (depth-1) Trainium2 kernel.

Full inputs: x [4, 4, 64, 256, 256] f32 + six banded Haar matrices
(hardcoded math: every output element is +-2^-1.5 times a +-sum of a
2x2x2 block). Returns the 8 subbands (LLL, LLH, LHL, LHH, HLL, HLH,
HHL, HHH), each [4, 4, 32, 128, 128] f32.

Sharding: data-parallel over N*C = 16 sample-channels, 2 per core on
8 cores. Per-core compute is a 3-stage butterfly over pair-packed
tiles (SBUF partition p holds input rows 2p and 2p+1 contiguously, so
every DMA descriptor is a 2 KiB linear run):
  H stage: TensorE float32r matmuls against +-2^-1.5 * I (1 cyc/row)
  evac:    ScalarE PSUM->SBUF fp16 copy that also de-interleaves
           even/odd w columns, so later DVE ops see packed fp16
  W stage: DVE fp16 tensor_add/sub, packed last dim -> 2x mode
  D stage: DVE fp16 tensor_add/sub on the two d-slices of the pair
Output is written fp16 (tolerance is 2e-2; fp16 error ~1e-3) which
halves the DMA write traffic; host upcasts to f32.
"""
import sys

sys.path.insert(0, "/opt/trn_rl_repo")

import numpy as np

N, C, D, H, W = 4, 4, 64, 256, 256
NCORES = 8
G_PER_CORE = (N * C) // NCORES        # 2
KP = D // 2                           # 32 d-pairs per g
KB = 8                                # k-slices per output staging block
S3 = float(2.0 ** -1.5)

IN_BUFS = 12
WE_BUFS = 4
WT_BUFS = 4
OS_BUFS = 2
PSUM_BUFS = 3

_CACHE = {}


def _build_filter_lhst():
    """Stationary operands: +S3*I and -S3*I, as [2, 128, 128] fp32."""
    eye = np.eye(128, dtype=np.float32)
    return np.stack([np.float32(S3) * eye, np.float32(-S3) * eye])


def _build_nc():
    import concourse.bass as bass
    import concourse.tile as tile
    from concourse import bacc, mybir

    f32 = mybir.dt.float32
    f32r = mybir.dt.float32r
    f16 = mybir.dt.float16
    nc = bacc.Bacc(None)
    x_d = nc.declare_dram_parameter("x", [G_PER_CORE, D, H, W], f32r,
                                    isOutput=False)
    ft_d = nc.declare_dram_parameter("ft", [2, 128, 128], f32r,
                                     isOutput=False)
    # h'-major fp16 layout: per (band, g, partition=h') a k-block of 8 is
    # one contiguous 2 KiB run in DRAM (host transposes k and h' back)
    o_d = nc.declare_dram_parameter("out", [8, G_PER_CORE, 128, KP, 128],
                                    f16, isOutput=True)

    with tile.TileContext(nc) as tc:
        with (
            tc.tile_pool(name="cst", bufs=1) as cst,
            tc.tile_pool(name="inp", bufs=IN_BUFS) as inp,
            tc.tile_pool(name="we", bufs=WE_BUFS) as wep,
            tc.tile_pool(name="wt", bufs=WT_BUFS) as wtp,
            tc.tile_pool(name="os", bufs=OS_BUFS) as osp,
            tc.tile_pool(name="ps", bufs=PSUM_BUFS, space="PSUM") as psp,
        ):
            ft = cst.tile([128, 256], f32r, tag="ft")
            nc.sync.dma_start(
                ft.rearrange("p (i c) -> p i c", i=2),
                ft_d.rearrange("i p c -> p i c"))
            pos_i = ft[:, 0:128]    # +S3 * I
            neg_i = ft[:, 128:256]  # -S3 * I

            for g in range(G_PER_CORE):
                for kb in range(KP // KB):
                    os_t = osp.tile([128, 8 * KB * 128], f16, tag="os")
                    # [p, band(8), k8(KB), w'(128)]
                    ot3 = os_t.rearrange("p (b q w) -> p b q w", b=8, q=KB)
                    for k8 in range(KB):
                        k = kb * KB + k8
                        # one d-pair as a pair-packed tile [128, 1024]:
                        # cols = {s0: row2p row2p+1 | s1: row2p row2p+1}
                        t = inp.tile([128, 1024], f32r, tag="xin")
                        nc.sync.dma_start(
                            t.rearrange("p (s r) -> p s r", s=2),
                            x_d[g, 2 * k:2 * k + 2].rearrange(
                                "s (p r) w -> p s (r w)", r=2))
                        t4 = t.rearrange("p (s r w) -> p s r w", s=2, r=2)

                        # --- H stage: float32r matmuls vs +-S3*I ---
                        pt = psp.tile([128, 1024], f32, tag="ps")
                        lo = pt[:, 0:512].rearrange(
                            "p (s w) -> p s w", s=2)
                        hi = pt[:, 512:1024].rearrange(
                            "p (s w) -> p s w", s=2)
                        x0 = t4[:, :, 0, :]
                        x1 = t4[:, :, 1, :]
                        nc.tensor.matmul(lo, pos_i, x0,
                                         start=True, stop=False)
                        nc.tensor.matmul(hi, pos_i, x0,
                                         start=True, stop=False)
                        nc.tensor.matmul(lo, pos_i, x1,
                                         start=False, stop=True)
                        nc.tensor.matmul(hi, neg_i, x1,
                                         start=False, stop=True)

                        # --- PSUM evac on ScalarE: f32 -> fp16 and
                        # de-interleave w parity for packed DVE reads ---
                        # WE: [p, lh(2), s(2), q(2 parity), w'(128)]
                        we_t = wep.tile([128, 1024], f16, tag="we")
                        we5 = we_t.rearrange("p (l s q w) -> p l s q w",
                                             l=2, s=2, q=2)
                        nc.scalar.activation(
                            we_t.rearrange("p (m q w) -> p m q w",
                                           m=4, q=2),
                            pt.rearrange("p (m w q) -> p m q w",
                                         m=4, q=2),
                            mybir.ActivationFunctionType.Copy)

                        # --- W stage on DVE, fp16 2x ---
                        # WT: [p, c(4 = lh*2+wp), s(2), w'(128)]
                        wt_t = wtp.tile([128, 1024], f16, tag="wt")
                        wt4 = wt_t.rearrange("p (c s w) -> p c s w",
                                             c=4, s=2)
                        in0 = we5[:, :, :, 0, :]
                        in1 = we5[:, :, :, 1, :]
                        nc.vector.tensor_add(wt4[:, 0::2], in0, in1)
                        nc.vector.tensor_sub(wt4[:, 1::2], in0, in1)

                        # --- D stage on DVE, fp16 2x ---
                        # band = dp*4 + c  (matches reference order)
                        s0 = wt4[:, :, 0, :]
                        s1 = wt4[:, :, 1, :]
                        nc.vector.tensor_add(ot3[:, 0:4, k8], s0, s1)
                        nc.vector.tensor_sub(ot3[:, 4:8, k8], s0, s1)

                    # --- store this k-block: 8 subbands x [128,KB,128].
                    # Issued from the (otherwise idle) GpSimd sequencer so
                    # they never block input prefetch on the Sync engine.
                    for b in range(8):
                        nc.gpsimd.dma_start(
                            o_d[b, g, :, kb * KB:(kb + 1) * KB, :],
                            ot3[:, b])
    nc.finalize()
    return nc


def _get_nc():
    if "nc" not in _CACHE:
        _CACHE["nc"] = _build_nc()
    return _CACHE["nc"]


def kernel(x, low_0, low_1, low_2, high_0, high_1, high_2):
    from concourse.bass_utils import run_bass_kernel_spmd

    x = np.ascontiguousarray(np.asarray(x, dtype=np.float32))
    ft = _build_filter_lhst()
    xs = x.reshape(N * C, D, H, W)
    in_maps = [
        {"x": np.ascontiguousarray(
            xs[c * G_PER_CORE:(c + 1) * G_PER_CORE]), "ft": ft}
        for c in range(NCORES)
    ]
    nc = _get_nc()
    res = run_bass_kernel_spmd(nc, in_maps, list(range(NCORES)))
    full = np.empty((8, N * C, KP, 128, 128), dtype=np.float32)
    for c in range(NCORES):
        full[:, c * G_PER_CORE:(c + 1) * G_PER_CORE] = \
            res.results[c]["out"].transpose(0, 1, 3, 2, 4).astype(
                np.float32)
    full = full.reshape(8, N, C, KP, 128, 128)
    return tuple(full[s] for s in range(8))


# revision 8
# speedup vs baseline: 1.2780x; 1.2094x over previous
"""3D Haar DWT (depth-1) Trainium2 kernel.

Full inputs: x [4, 4, 64, 256, 256] f32 + six banded Haar matrices
(hardcoded math: every output element is +-2^-1.5 times a +-sum of a
2x2x2 block). Returns the 8 subbands (LLL, LLH, LHL, LHH, HLL, HLH,
HHL, HHH), each [4, 4, 32, 128, 128] f32.

Sharding: data-parallel over N*C = 16 sample-channels, 2 per core on
8 cores. Per-core compute is a 3-stage butterfly over pair-packed
tiles (SBUF partition p holds input rows 2p and 2p+1 contiguously, so
every DMA descriptor is a 2 KiB linear run):
  H stage: TensorE float32r matmuls against +-2^-1.5 * I (1 cyc/row)
  evac:    ScalarE PSUM->SBUF fp16 copy that also de-interleaves
           even/odd w columns, so later DVE ops see packed fp16
  W stage: DVE fp16 tensor_add/sub, packed last dim -> 2x mode
  D stage: DVE fp16 tensor_add/sub on the two d-slices of the pair
Output is written fp16 (tolerance is 2e-2; fp16 error ~1e-3) which
halves the DMA write traffic; host upcasts to f32.
"""
import sys

sys.path.insert(0, "/opt/trn_rl_repo")

import numpy as np

N, C, D, H, W = 4, 4, 64, 256, 256
NCORES = 8
G_PER_CORE = (N * C) // NCORES        # 2
KP = D // 2                           # 32 d-pairs per g
KB = 8                                # k-slices per output staging block
S3 = float(2.0 ** -1.5)

IN_BUFS = 12
WE_BUFS = 6
WT_BUFS = 6
OS_BUFS = 3
PSUM_BUFS = 3

_CACHE = {}


def _build_filter_lhst():
    """Stationary operands: +S3*I and -S3*I, as [2, 128, 128] fp32."""
    eye = np.eye(128, dtype=np.float32)
    return np.stack([np.float32(S3) * eye, np.float32(-S3) * eye])


def _build_nc():
    import concourse.bass as bass
    import concourse.tile as tile
    from concourse import bacc, mybir

    f32 = mybir.dt.float32
    f32r = mybir.dt.float32r
    f16 = mybir.dt.float16
    nc = bacc.Bacc(None)
    x_d = nc.declare_dram_parameter("x", [G_PER_CORE, D, H, W], f32r,
                                    isOutput=False)
    ft_d = nc.declare_dram_parameter("ft", [2, 128, 128], f32r,
                                     isOutput=False)
    # h'-major fp16 layout: per (band, g, partition=h') a k-block of 8 is
    # one contiguous 2 KiB run in DRAM (host transposes k and h' back)
    o_d = nc.declare_dram_parameter("out", [8, G_PER_CORE, 128, KP, 128],
                                    f16, isOutput=True)

    with tile.TileContext(nc) as tc:
        with (
            tc.tile_pool(name="cst", bufs=1) as cst,
            tc.tile_pool(name="inp", bufs=IN_BUFS) as inp,
            tc.tile_pool(name="we", bufs=WE_BUFS) as wep,
            tc.tile_pool(name="wt", bufs=WT_BUFS) as wtp,
            tc.tile_pool(name="os", bufs=OS_BUFS) as osp,
            tc.tile_pool(name="ps", bufs=PSUM_BUFS, space="PSUM") as psp,
        ):
            ft = cst.tile([128, 256], f32r, tag="ft")
            nc.sync.dma_start(
                ft.rearrange("p (i c) -> p i c", i=2),
                ft_d.rearrange("i p c -> p i c"))
            pos_i = ft[:, 0:128]    # +S3 * I
            neg_i = ft[:, 128:256]  # -S3 * I

            for g in range(G_PER_CORE):
                for kb in range(KP // KB):
                    os_t = osp.tile([128, 8 * KB * 128], f16, tag="os")
                    # [p, band(8), k8(KB), w'(128)]
                    ot3 = os_t.rearrange("p (b q w) -> p b q w", b=8, q=KB)
                    for k8 in range(KB):
                        k = kb * KB + k8
                        # one d-pair as a pair-packed tile [128, 1024]:
                        # cols = {s0: row2p row2p+1 | s1: row2p row2p+1}
                        t = inp.tile([128, 1024], f32r, tag="xin")
                        nc.sync.dma_start(
                            t.rearrange("p (s r) -> p s r", s=2),
                            x_d[g, 2 * k:2 * k + 2].rearrange(
                                "s (p r) w -> p s (r w)", r=2))
                        t4 = t.rearrange("p (s r w) -> p s r w", s=2, r=2)

                        # --- H stage: float32r matmuls vs +-S3*I ---
                        pt = psp.tile([128, 1024], f32, tag="ps")
                        lo = pt[:, 0:512].rearrange(
                            "p (s w) -> p s w", s=2)
                        hi = pt[:, 512:1024].rearrange(
                            "p (s w) -> p s w", s=2)
                        x0 = t4[:, :, 0, :]
                        x1 = t4[:, :, 1, :]
                        nc.tensor.matmul(lo, pos_i, x0,
                                         start=True, stop=False)
                        nc.tensor.matmul(hi, pos_i, x0,
                                         start=True, stop=False)
                        nc.tensor.matmul(lo, pos_i, x1,
                                         start=False, stop=True)
                        nc.tensor.matmul(hi, neg_i, x1,
                                         start=False, stop=True)

                        # --- PSUM evac on ScalarE: f32 -> fp16 and
                        # de-interleave w parity for packed DVE reads ---
                        # WE: [p, lh(2), s(2), q(2 parity), w'(128)]
                        we_t = wep.tile([128, 1024], f16, tag="we")
                        we5 = we_t.rearrange("p (l s q w) -> p l s q w",
                                             l=2, s=2, q=2)
                        nc.scalar.activation(
                            we_t.rearrange("p (m q w) -> p m q w",
                                           m=4, q=2),
                            pt.rearrange("p (m w q) -> p m q w",
                                         m=4, q=2),
                            mybir.ActivationFunctionType.Copy)

                        # --- W stage on DVE, fp16 2x ---
                        # WT: [p, c(4 = lh*2+wp), s(2), w'(128)]
                        wt_t = wtp.tile([128, 1024], f16, tag="wt")
                        wt4 = wt_t.rearrange("p (c s w) -> p c s w",
                                             c=4, s=2)
                        in0 = we5[:, :, :, 0, :]
                        in1 = we5[:, :, :, 1, :]
                        nc.vector.tensor_add(wt4[:, 0::2], in0, in1)
                        nc.vector.tensor_sub(wt4[:, 1::2], in0, in1)

                        # --- D stage on DVE, fp16 2x ---
                        # band = dp*4 + c  (matches reference order)
                        s0 = wt4[:, :, 0, :]
                        s1 = wt4[:, :, 1, :]
                        nc.vector.tensor_add(ot3[:, 0:4, k8], s0, s1)
                        nc.vector.tensor_sub(ot3[:, 4:8, k8], s0, s1)

                    # --- store this k-block: 8 subbands x [128,KB,128].
                    # Split across two DMA queues: 4 bands on the GpSimd
                    # software DGE (~130 GB/s cap) and 4 on the Scalar
                    # hardware DGE queue; neither issuer blocks input
                    # prefetch on the Sync engine, and the Scalar issue
                    # only stalls ~2us/block (ACT runs just ahead of DVE).
                    for b in range(8):
                        eng = nc.gpsimd if b < 4 else nc.scalar
                        eng.dma_start(
                            o_d[b, g, :, kb * KB:(kb + 1) * KB, :],
                            ot3[:, b])
    nc.finalize()
    return nc


def _get_nc():
    if "nc" not in _CACHE:
        _CACHE["nc"] = _build_nc()
    return _CACHE["nc"]


def kernel(x, low_0, low_1, low_2, high_0, high_1, high_2):
    from concourse.bass_utils import run_bass_kernel_spmd

    x = np.ascontiguousarray(np.asarray(x, dtype=np.float32))
    ft = _build_filter_lhst()
    xs = x.reshape(N * C, D, H, W)
    in_maps = [
        {"x": np.ascontiguousarray(
            xs[c * G_PER_CORE:(c + 1) * G_PER_CORE]), "ft": ft}
        for c in range(NCORES)
    ]
    nc = _get_nc()
    res = run_bass_kernel_spmd(nc, in_maps, list(range(NCORES)))
    full = np.empty((8, N * C, KP, 128, 128), dtype=np.float32)
    for c in range(NCORES):
        full[:, c * G_PER_CORE:(c + 1) * G_PER_CORE] = \
            res.results[c]["out"].transpose(0, 1, 3, 2, 4).astype(
                np.float32)
    full = full.reshape(8, N, C, KP, 128, 128)
    return tuple(full[s] for s in range(8))


# revision 9
# speedup vs baseline: 1.2922x; 1.0111x over previous
"""3D Haar DWT (depth-1) Trainium2 kernel.

Full inputs: x [4, 4, 64, 256, 256] f32 + six banded Haar matrices
(hardcoded math: every output element is +-2^-1.5 times a +-sum of a
2x2x2 block). Returns the 8 subbands (LLL, LLH, LHL, LHH, HLL, HLH,
HHL, HHH), each [4, 4, 32, 128, 128] f32.

Sharding: data-parallel over N*C = 16 sample-channels, 2 per core on
8 cores. Per-core compute is a 3-stage butterfly over pair-packed
tiles (SBUF partition p holds input rows 2p and 2p+1 contiguously, so
every DMA descriptor is a 2 KiB linear run):
  H stage: TensorE float32r matmuls against +-2^-1.5 * I (1 cyc/row)
  evac:    ScalarE PSUM->SBUF fp16 copy that also de-interleaves
           even/odd w columns, so later DVE ops see packed fp16
  W stage: DVE fp16 tensor_add/sub, packed last dim -> 2x mode
  D stage: DVE fp16 tensor_add/sub on the two d-slices of the pair
Output is written fp16 (tolerance is 2e-2; fp16 error ~1e-3) which
halves the DMA write traffic; host upcasts to f32.
"""
import sys

sys.path.insert(0, "/opt/trn_rl_repo")

import numpy as np

N, C, D, H, W = 4, 4, 64, 256, 256
NCORES = 8
G_PER_CORE = (N * C) // NCORES        # 2
KP = D // 2                           # 32 d-pairs per g
KB = 8                                # k-slices per output staging block
S3 = float(2.0 ** -1.5)

IN_BUFS = 12
WE_BUFS = 6
WT_BUFS = 6
OS_BUFS = 3
PSUM_BUFS = 3

_CACHE = {}


def _build_filter_lhst():
    """Stationary operands: +S3*I and -S3*I, as [2, 128, 128] fp32."""
    eye = np.eye(128, dtype=np.float32)
    return np.stack([np.float32(S3) * eye, np.float32(-S3) * eye])


def _build_nc():
    import concourse.bass as bass
    import concourse.tile as tile
    from concourse import bacc, mybir

    f32 = mybir.dt.float32
    f32r = mybir.dt.float32r
    f16 = mybir.dt.float16
    nc = bacc.Bacc(None)
    x_d = nc.declare_dram_parameter("x", [G_PER_CORE, D, H, W], f32r,
                                    isOutput=False)
    ft_d = nc.declare_dram_parameter("ft", [2, 128, 128], f32r,
                                     isOutput=False)
    # h'-major fp16 layout: per (band, g, partition=h') a k-block of 8 is
    # one contiguous 2 KiB run in DRAM (host transposes k and h' back)
    o_d = nc.declare_dram_parameter("out", [8, G_PER_CORE, 128, KP, 128],
                                    f16, isOutput=True)

    with tile.TileContext(nc) as tc:
        with (
            tc.tile_pool(name="cst", bufs=1) as cst,
            tc.tile_pool(name="inp", bufs=IN_BUFS) as inp,
            tc.tile_pool(name="we", bufs=WE_BUFS) as wep,
            tc.tile_pool(name="wt", bufs=WT_BUFS) as wtp,
            tc.tile_pool(name="os", bufs=OS_BUFS) as osp,
            tc.tile_pool(name="ps", bufs=PSUM_BUFS, space="PSUM") as psp,
        ):
            ft = cst.tile([128, 256], f32r, tag="ft")
            nc.sync.dma_start(
                ft.rearrange("p (i c) -> p i c", i=2),
                ft_d.rearrange("i p c -> p i c"))
            pos_i = ft[:, 0:128]    # +S3 * I
            neg_i = ft[:, 128:256]  # -S3 * I

            for g in range(G_PER_CORE):
                for kb in range(KP // KB):
                    os_t = osp.tile([128, 8 * KB * 128], f16, tag="os")
                    # [p, band(8), k8(KB), w'(128)]
                    ot3 = os_t.rearrange("p (b q w) -> p b q w", b=8, q=KB)
                    for k8 in range(KB):
                        k = kb * KB + k8
                        # one d-pair as a pair-packed tile [128, 1024]:
                        # cols = {s0: row2p row2p+1 | s1: row2p row2p+1}
                        t = inp.tile([128, 1024], f32r, tag="xin")
                        nc.sync.dma_start(
                            t.rearrange("p (s r) -> p s r", s=2),
                            x_d[g, 2 * k:2 * k + 2].rearrange(
                                "s (p r) w -> p s (r w)", r=2))
                        t4 = t.rearrange("p (s r w) -> p s r w", s=2, r=2)

                        # --- H stage: float32r matmuls vs +-S3*I ---
                        pt = psp.tile([128, 1024], f32, tag="ps")
                        lo = pt[:, 0:512].rearrange(
                            "p (s w) -> p s w", s=2)
                        hi = pt[:, 512:1024].rearrange(
                            "p (s w) -> p s w", s=2)
                        x0 = t4[:, :, 0, :]
                        x1 = t4[:, :, 1, :]
                        nc.tensor.matmul(lo, pos_i, x0,
                                         start=True, stop=False)
                        nc.tensor.matmul(hi, pos_i, x0,
                                         start=True, stop=False)
                        nc.tensor.matmul(lo, pos_i, x1,
                                         start=False, stop=True)
                        nc.tensor.matmul(hi, neg_i, x1,
                                         start=False, stop=True)

                        # --- PSUM evac on ScalarE: f32 -> fp16 and
                        # de-interleave w parity for packed DVE reads ---
                        # WE: [p, lh(2), s(2), q(2 parity), w'(128)]
                        we_t = wep.tile([128, 1024], f16, tag="we")
                        we5 = we_t.rearrange("p (l s q w) -> p l s q w",
                                             l=2, s=2, q=2)
                        nc.scalar.activation(
                            we_t.rearrange("p (m q w) -> p m q w",
                                           m=4, q=2),
                            pt.rearrange("p (m w q) -> p m q w",
                                         m=4, q=2),
                            mybir.ActivationFunctionType.Copy)

                        # --- W stage on DVE, fp16 2x ---
                        # WT: [p, c(4 = lh*2+wp), s(2), w'(128)]
                        wt_t = wtp.tile([128, 1024], f16, tag="wt")
                        wt4 = wt_t.rearrange("p (c s w) -> p c s w",
                                             c=4, s=2)
                        in0 = we5[:, :, :, 0, :]
                        in1 = we5[:, :, :, 1, :]
                        nc.vector.tensor_add(wt4[:, 0::2], in0, in1)
                        nc.vector.tensor_sub(wt4[:, 1::2], in0, in1)

                        # --- D stage on DVE, fp16 2x ---
                        # band = dp*4 + c  (matches reference order)
                        s0 = wt4[:, :, 0, :]
                        s1 = wt4[:, :, 1, :]
                        nc.vector.tensor_add(ot3[:, 0:4, k8], s0, s1)
                        nc.vector.tensor_sub(ot3[:, 4:8, k8], s0, s1)

                    # --- store this k-block: 8 subbands x [128,KB,128].
                    # Issued from the (otherwise idle) GpSimd sequencer so
                    # they never block input prefetch on the Sync engine;
                    # 2 KiB runs keep the SW DGE queue at full rate.
                    for b in range(8):
                        nc.gpsimd.dma_start(
                            o_d[b, g, :, kb * KB:(kb + 1) * KB, :],
                            ot3[:, b])
    nc.finalize()
    return nc


def _get_nc():
    if "nc" not in _CACHE:
        _CACHE["nc"] = _build_nc()
    return _CACHE["nc"]


def kernel(x, low_0, low_1, low_2, high_0, high_1, high_2):
    from concourse.bass_utils import run_bass_kernel_spmd

    x = np.ascontiguousarray(np.asarray(x, dtype=np.float32))
    ft = _build_filter_lhst()
    xs = x.reshape(N * C, D, H, W)
    in_maps = [
        {"x": np.ascontiguousarray(
            xs[c * G_PER_CORE:(c + 1) * G_PER_CORE]), "ft": ft}
        for c in range(NCORES)
    ]
    nc = _get_nc()
    res = run_bass_kernel_spmd(nc, in_maps, list(range(NCORES)))
    full = np.empty((8, N * C, KP, 128, 128), dtype=np.float32)
    for c in range(NCORES):
        full[:, c * G_PER_CORE:(c + 1) * G_PER_CORE] = \
            res.results[c]["out"].transpose(0, 1, 3, 2, 4).astype(
                np.float32)
    full = full.reshape(8, N, C, KP, 128, 128)
    return tuple(full[s] for s in range(8))


# revision 10
# speedup vs baseline: 1.3522x; 1.0465x over previous
"""3D Haar DWT (depth-1) Trainium2 kernel.

Full inputs: x [4, 4, 64, 256, 256] f32 + six banded Haar matrices
(hardcoded math: every output element is +-2^-1.5 times a +-sum of a
2x2x2 block). Returns the 8 subbands (LLL, LLH, LHL, LHH, HLL, HLH,
HHL, HHH), each [4, 4, 32, 128, 128] f32.

Sharding: data-parallel over N*C = 16 sample-channels, 2 per core on
8 cores. Per-core compute is a 3-stage butterfly over pair-packed
tiles (SBUF partition p holds input rows 2p and 2p+1 contiguously, so
every DMA descriptor is a 2 KiB linear run):
  H stage: TensorE float32r matmuls against +-2^-1.5 * I (1 cyc/row)
  evac:    ScalarE PSUM->SBUF fp16 copy that also de-interleaves
           even/odd w columns, so later DVE ops see packed fp16
  W stage: DVE fp16 tensor_add/sub, packed last dim -> 2x mode
  D stage: DVE fp16 tensor_add/sub on the two d-slices of the pair
Output is written fp16 (tolerance is 2e-2; fp16 error ~1e-3) which
halves the DMA write traffic; host upcasts to f32.
"""
import sys

sys.path.insert(0, "/opt/trn_rl_repo")

import numpy as np

N, C, D, H, W = 4, 4, 64, 256, 256
NCORES = 8
G_PER_CORE = (N * C) // NCORES        # 2
KP = D // 2                           # 32 d-pairs per g
KB = 4                                # k-slices per output staging block
S3 = float(2.0 ** -1.5)

IN_BUFS = 12
WE_BUFS = 6
WT_BUFS = 6
OS_BUFS = 4
PSUM_BUFS = 4

_CACHE = {}


def _build_filter_lhst():
    """Stationary operands: +S3*I and -S3*I, as [2, 128, 128] fp32."""
    eye = np.eye(128, dtype=np.float32)
    return np.stack([np.float32(S3) * eye, np.float32(-S3) * eye])


def _build_nc():
    import concourse.bass as bass
    import concourse.tile as tile
    from concourse import bacc, mybir

    f32 = mybir.dt.float32
    f32r = mybir.dt.float32r
    f16 = mybir.dt.float16
    nc = bacc.Bacc(None)
    x_d = nc.declare_dram_parameter("x", [G_PER_CORE, D, H, W], f32r,
                                    isOutput=False)
    ft_d = nc.declare_dram_parameter("ft", [2, 128, 128], f32r,
                                     isOutput=False)
    # h'-major fp16 layout: per (band, g, partition=h') a k-block of 8 is
    # one contiguous 2 KiB run in DRAM (host transposes k and h' back)
    o_d = nc.declare_dram_parameter("out", [8, G_PER_CORE, 128, KP, 128],
                                    f16, isOutput=True)

    with tile.TileContext(nc) as tc:
        with (
            tc.tile_pool(name="cst", bufs=1) as cst,
            tc.tile_pool(name="inp", bufs=IN_BUFS) as inp,
            tc.tile_pool(name="we", bufs=WE_BUFS) as wep,
            tc.tile_pool(name="wt", bufs=WT_BUFS) as wtp,
            tc.tile_pool(name="os", bufs=OS_BUFS) as osp,
            tc.tile_pool(name="ps", bufs=PSUM_BUFS, space="PSUM") as psp,
        ):
            ft = cst.tile([128, 256], f32r, tag="ft")
            nc.sync.dma_start(
                ft.rearrange("p (i c) -> p i c", i=2),
                ft_d.rearrange("i p c -> p i c"))
            pos_i = ft[:, 0:128]    # +S3 * I
            neg_i = ft[:, 128:256]  # -S3 * I

            for g in range(G_PER_CORE):
                for kb in range(KP // KB):
                    os_t = osp.tile([128, 8 * KB * 128], f16, tag="os")
                    # [p, band(8), k8(KB), w'(128)]
                    ot3 = os_t.rearrange("p (b q w) -> p b q w", b=8, q=KB)
                    for k8 in range(KB):
                        k = kb * KB + k8
                        # one d-pair as a pair-packed tile [128, 1024]:
                        # cols = {s0: row2p row2p+1 | s1: row2p row2p+1}
                        t = inp.tile([128, 1024], f32r, tag="xin")
                        nc.sync.dma_start(
                            t.rearrange("p (s r) -> p s r", s=2),
                            x_d[g, 2 * k:2 * k + 2].rearrange(
                                "s (p r) w -> p s (r w)", r=2))
                        t4 = t.rearrange("p (s r w) -> p s r w", s=2, r=2)

                        # --- H stage: float32r matmuls vs +-S3*I ---
                        pt = psp.tile([128, 1024], f32, tag="ps")
                        lo = pt[:, 0:512].rearrange(
                            "p (s w) -> p s w", s=2)
                        hi = pt[:, 512:1024].rearrange(
                            "p (s w) -> p s w", s=2)
                        x0 = t4[:, :, 0, :]
                        x1 = t4[:, :, 1, :]
                        nc.tensor.matmul(lo, pos_i, x0,
                                         start=True, stop=False)
                        nc.tensor.matmul(hi, pos_i, x0,
                                         start=True, stop=False)
                        nc.tensor.matmul(lo, pos_i, x1,
                                         start=False, stop=True)
                        nc.tensor.matmul(hi, neg_i, x1,
                                         start=False, stop=True)

                        # --- PSUM evac on ScalarE: f32 -> fp16 and
                        # de-interleave w parity for packed DVE reads ---
                        # WE: [p, lh(2), s(2), q(2 parity), w'(128)]
                        we_t = wep.tile([128, 1024], f16, tag="we")
                        we5 = we_t.rearrange("p (l s q w) -> p l s q w",
                                             l=2, s=2, q=2)
                        for lh, psrc in ((0, pt[:, 0:512]),
                                         (1, pt[:, 512:1024])):
                            nc.scalar.activation(
                                we5[:, lh],
                                psrc.rearrange("p (s w q) -> p s q w",
                                               s=2, q=2),
                                mybir.ActivationFunctionType.Copy)

                        # --- W stage on DVE, fp16 2x ---
                        # WT: [p, c(4 = lh*2+wp), s(2), w'(128)]
                        wt_t = wtp.tile([128, 1024], f16, tag="wt")
                        wt4 = wt_t.rearrange("p (c s w) -> p c s w",
                                             c=4, s=2)
                        in0 = we5[:, :, :, 0, :]
                        in1 = we5[:, :, :, 1, :]
                        nc.vector.tensor_add(wt4[:, 0::2], in0, in1)
                        nc.vector.tensor_sub(wt4[:, 1::2], in0, in1)

                        # --- D stage on DVE, fp16 2x ---
                        # band = dp*4 + c  (matches reference order)
                        s0 = wt4[:, :, 0, :]
                        s1 = wt4[:, :, 1, :]
                        nc.vector.tensor_add(ot3[:, 0:4, k8], s0, s1)
                        nc.vector.tensor_sub(ot3[:, 4:8, k8], s0, s1)

                    # --- store this k-block: 8 subbands x [128,KB,128].
                    # Issued from the (otherwise idle) GpSimd sequencer so
                    # they never block input prefetch on the Sync engine.
                    for b in range(8):
                        nc.gpsimd.dma_start(
                            o_d[b, g, :, kb * KB:(kb + 1) * KB, :],
                            ot3[:, b])
    nc.finalize()
    return nc


def _get_nc():
    if "nc" not in _CACHE:
        _CACHE["nc"] = _build_nc()
    return _CACHE["nc"]


def kernel(x, low_0, low_1, low_2, high_0, high_1, high_2):
    from concourse.bass_utils import run_bass_kernel_spmd

    x = np.ascontiguousarray(np.asarray(x, dtype=np.float32))
    ft = _build_filter_lhst()
    xs = x.reshape(N * C, D, H, W)
    in_maps = [
        {"x": np.ascontiguousarray(
            xs[c * G_PER_CORE:(c + 1) * G_PER_CORE]), "ft": ft}
        for c in range(NCORES)
    ]
    nc = _get_nc()
    res = run_bass_kernel_spmd(nc, in_maps, list(range(NCORES)))
    full = np.empty((8, N * C, KP, 128, 128), dtype=np.float32)
    for c in range(NCORES):
        full[:, c * G_PER_CORE:(c + 1) * G_PER_CORE] = \
            res.results[c]["out"].transpose(0, 1, 3, 2, 4).astype(
                np.float32)
    full = full.reshape(8, N, C, KP, 128, 128)
    return tuple(full[s] for s in range(8))
